# revision 5
# baseline (speedup 1.0000x reference)
"""Trainium2 Bass kernel v3: WOQ Linear -> +add1+add2 -> WOQ Linear -> mul.

v4 = v3 + single qweight load: layer 1 adopts layer 2's contraction tiling
(k-tile g holds W rows pi[128g..128g+128) = n0+4p, a single stride-4 row
gather), so the packed weights load once and both layers share one scale
array. Also: first scale tile ahead of the boot queue; last super's output
DMAs on the gpsimd queue to shorten the tail.

v3 was v2 with kt-pair batching of the dequant pipeline:
 - extract per kt-pair (g, g+8): one tensor_scalar over [128,2,256] i32
   (amortizes the ~280ns DVE fixed cost measured on HW)
 - cast on ACT from CONTIGUOUS i16 lanes (no stride-2 u8 penalty):
   nib2 [128,512]i32 -> bitcast i16 [128,1024] -> bf16
 - mult per kt-pair: [128,1024] TT with a paired scale tile (same variant
   for g and g+8 by construction)
 - 8 matmuls per (pair, super)
Everything else (pi layout, group-interleaved layer-1 tiling, correction
matmul, resident ar_b, in-place qw2 reload) as v2.
"""

import numpy as np
import ml_dtypes

import concourse.bass as bass  # noqa: F401
from concourse import bacc
import concourse.tile as tile
import concourse.mybir as mybir
from concourse.alu_op_type import AluOpType
from contextlib import ExitStack

BF16 = mybir.dt.bfloat16
F32 = mybir.dt.float32
F32R = mybir.dt.float32r
I32 = mybir.dt.int32
I16 = mybir.dt.int16
BF = ml_dtypes.bfloat16

D = 4096
GS = 128
NPK = 512
G_N = 32
EC = G_N + 1
T_CORE = 512
N_CORES = 8
NSUP = 8
SW = 512

# kt pairs (g, g+8), same scale variant in both layers
PAIRS = [(a, a + 8) for a in list(range(0, 8)) + list(range(16, 24))]


def make_pi(d=D):
    pos = np.arange(d)
    s = pos // SW
    c = pos % SW
    return 2048 * (s % 2) + 8 * (c // 2) + (s // 2) + 4 * (c % 2)


def k_perm(d=D):
    g1 = np.arange(d) // 128
    p = np.arange(d) % 128
    return 1024 * (g1 % 4) + 8 * p + (g1 // 4)


def build_program(t=T_CORE):
    nc = bacc.Bacc()
    qw_d = nc.dram_tensor("qweight", [D, NPK], I32, kind="ExternalInput")
    xt_d = nc.dram_tensor("xt_bf", [D, t], BF16, kind="ExternalInput")
    sb_d = nc.dram_tensor("sb", [NSUP * 8 * 128, SW], BF16, kind="ExternalInput")
    c_d = nc.dram_tensor("c_mat", [EC, D], F32R, kind="ExternalInput")
    r1_d = nc.dram_tensor("r1", [EC, t], F32R, kind="ExternalInput")
    e2_d = nc.dram_tensor("e2", [8 * 128, EC], BF16, kind="ExternalInput")
    a12_d = nc.dram_tensor("a12t", [D, t], BF16, kind="ExternalInput")
    a1_d = nc.dram_tensor("a1t", [D, t], BF16, kind="ExternalInput")
    out_d = nc.dram_tensor("outt", [D, t], BF16, kind="ExternalOutput")

    with tile.TileContext(nc) as tc, ExitStack() as ctx:
        const = ctx.enter_context(tc.tile_pool(name="const", bufs=1))
        resid = ctx.enter_context(tc.tile_pool(name="resid", bufs=1))
        scp = ctx.enter_context(tc.tile_pool(name="scp", bufs=10))
        nibp = ctx.enter_context(tc.tile_pool(name="nibp", bufs=3))
        nbfp = ctx.enter_context(tc.tile_pool(name="nbfp", bufs=3))
        wp = ctx.enter_context(tc.tile_pool(name="wp", bufs=3))
        avp = ctx.enter_context(tc.tile_pool(name="avp", bufs=8))
        yp = ctx.enter_context(tc.tile_pool(name="yp", bufs=3))
        outp = ctx.enter_context(tc.tile_pool(name="outp", bufs=4))
        psp = ctx.enter_context(tc.tile_pool(name="psp", bufs=8, space="PSUM"))

        # small residents (DMAs issued lazily mid-super-0 to keep the boot
        # DMA queue clear for the first weight tiles)
        c_sb = const.tile([EC, D], F32R)
        e2_sb = const.tile([128, 8 * EC], BF16)
        r1s = const.tile([EC, t], F32R)
        r2f = const.tile([EC, t], F32)
        r2s = const.tile([EC, t], F32R)

        xt_sb = resid.tile([128, 32 * t], BF16)
        ar_b = resid.tile([128, 32 * t], BF16)
        qw_res = resid.tile([128, 32 * NPK], I32)
        qw_v = qw_res[:].rearrange("p (G c) -> p G c", c=NPK)

        for layer in (1, 2):
            r_sb = r1s if layer == 1 else r2s
            nv = 8
            for s in range(NSUP):
                jj, hh = s // 2, s % 2
                scs = []
                for v in range(nv):
                    sc = scp.tile([128, 2 * SW], BF16, tag="sc",
                                  name=f"sc_{layer}_{s}_{v}")
                    src = sb_d[(s * nv + v) * 128:(s * nv + v + 1) * 128, :]
                    nc.sync.dma_start(sc[:, 0:SW], src)
                    nc.sync.dma_start(sc[:, SW:2 * SW], src)
                    scs.append(sc)
                ps = [psp.tile([128, t], F32, tag="ps",
                               name=f"ps_{layer}_{s}_{b}") for b in range(4)]
                for pidx, (g0, g1) in enumerate(PAIRS):
                    if s == 0 and layer == 1:
                        for g in (g0, g1):
                            sB, bB = g // 4, g % 4
                            n0 = 2048 * (sB % 2) + 512 * bB + sB // 2
                            nc.sync.dma_start(
                                qw_res[:, g * NPK:(g + 1) * NPK],
                                qw_d[n0:n0 + 4 * 127 + 1:4, :])
                            nc.sync.dma_start(
                                xt_sb[:, g * t:(g + 1) * t],
                                xt_d[g * 128:(g + 1) * 128, :])
                        if pidx == 4:
                            # queue small residents behind the first tiles
                            nc.sync.dma_start(c_sb[:], c_d[:])
                            nc.sync.dma_start(
                                e2_sb[:].rearrange("p (v e) -> p v e", e=EC),
                                e2_d[:].rearrange("(v p) e -> p v e", p=128))
                            nc.sync.dma_start(r1s[:], r1_d[:])
                    qs = qw_v[:, g0:g0 + 9:8, 256 * hh:256 * hh + 256]
                    nib = nibp.tile([128, SW], I32, tag="nib",
                                    name=f"nib_{layer}_{s}_{pidx}")
                    nc.vector.tensor_scalar(
                        nib[:].rearrange("p (a c) -> p a c", a=2), qs,
                        4 * jj, 0x000F000F,
                        AluOpType.logical_shift_right, AluOpType.bitwise_and)
                    nbf = nbfp.tile([128, 2 * SW], BF16, tag="nbf",
                                    name=f"nbf_{layer}_{s}_{pidx}")
                    nc.scalar.copy(nbf[:], nib[:].bitcast(I16))
                    w_t = wp.tile([128, 2 * SW], BF16, tag="w",
                                  name=f"w_{layer}_{s}_{pidx}")
                    v = 4 * ((g0 // 4) % 2) + (g0 % 4)
                    nc.vector.tensor_tensor(w_t[:], nbf[:], scs[v][:],
                                            AluOpType.mult)
                    for i, g in enumerate((g0, g1)):
                        rhs = (xt_sb if layer == 1 else ar_b)[:, g * t:(g + 1) * t]
                        for b in range(4):
                            nc.tensor.matmul(
                                ps[b][:],
                                w_t[:, i * SW + b * 128:i * SW + (b + 1) * 128],
                                rhs, start=(g == 0), stop=False)
                for b in range(4):
                    nc.tensor.matmul(
                        ps[b][:], c_sb[:, s * SW + b * 128:s * SW + (b + 1) * 128],
                        r_sb[:], start=False, stop=True)
                for b in range(4):
                    g2 = 4 * s + b
                    if layer == 1:
                        a12t = avp.tile([128, t], BF16, tag="av",
                                        name=f"a12_{s}_{b}")
                        nc.sync.dma_start(a12t[:],
                                          a12_d[g2 * 128:(g2 + 1) * 128, :])
                        nc.vector.tensor_tensor(ar_b[:, g2 * t:(g2 + 1) * t],
                                                ps[b][:], a12t[:], AluOpType.add)
                    else:
                        a1t = avp.tile([128, t], BF16, tag="av",
                                       name=f"a1_{s}_{b}")
                        nc.sync.dma_start(a1t[:],
                                          a1_d[g2 * 128:(g2 + 1) * 128, :])
                        y1 = yp.tile([128, t], F32, tag="y", name=f"y_{s}_{b}")
                        nc.vector.tensor_tensor(y1[:], ps[b][:], a1t[:],
                                                AluOpType.add)
                        ot = outp.tile([128, t], BF16, tag="ot",
                                       name=f"ot_{s}_{b}")
                        nc.vector.tensor_tensor(ot[:], y1[:],
                                                ar_b[:, g2 * t:(g2 + 1) * t],
                                                AluOpType.mult)
                        dma_eng = nc.gpsimd if s == NSUP - 1 else nc.sync
                        dma_eng.dma_start(out_d[g2 * 128:(g2 + 1) * 128, :],
                                          ot[:])
            if layer == 1:
                ps_r = psp.tile([128, t], F32, tag="ps", name="ps_r")
                for g2 in range(32):
                    hb = 4 * ((g2 // 4) % 2) + (g2 % 4)
                    nc.tensor.matmul(ps_r[0:EC, :],
                                     e2_sb[:, hb * EC:(hb + 1) * EC],
                                     ar_b[:, g2 * t:(g2 + 1) * t],
                                     start=(g2 == 0), stop=(g2 == 31))
                nc.vector.memset(r2f[:], 1.0)
                nc.vector.tensor_copy(r2f[0:G_N, :], ps_r[0:G_N, :])
                nc.vector.tensor_copy(r2s[:], r2f[:])
    nc.compile()
    return nc


def host_prep(inp, qweight, woq_scales, woq_qzeros, woq_bias, add1, add2,
              t=T_CORE, n_cores=N_CORES):
    pi = make_pi()
    kp = k_perm()
    x = inp.reshape(-1, D)
    a1 = add1.reshape(-1, D)
    a12 = (a1 + add2.reshape(-1, D))

    shifts = (np.arange(8, dtype=np.int32) * 4)
    z = ((woq_qzeros[:, :, None] >> shifts) & 0xF).reshape(G_N, D).astype(np.float32)
    zs = z * woq_scales
    c_mat = np.empty((EC, D), dtype=np.float32)
    c_mat[:G_N] = -zs[:, pi]
    c_mat[G_N] = woq_bias[pi]

    s_bf = woq_scales.astype(BF)
    pi_cols = pi.reshape(NSUP, SW)
    hbi = np.arange(8)
    G0 = 16 * (hbi // 4) + 4 * (hbi % 4)
    g2_row = G0[:, None] + np.arange(128)[None, :] // 32
    sb = s_bf[g2_row[None, :, :, None], pi_cols[:, None, None, :]]
    e2b = np.zeros((8, 128, EC), dtype=BF)
    e2b[hbi[:, None], np.arange(128)[None, :], g2_row] = 1

    in_maps = []
    for i in range(n_cores):
        sl = slice(i * t, (i + 1) * t)
        xtb_nat = np.ascontiguousarray(x[sl].T).astype(BF)
        r1 = np.ones((EC, t), dtype=np.float32)
        r1[:G_N] = xtb_nat.astype(np.float32).reshape(G_N, GS, t).sum(axis=1)
        in_maps.append({
            "qweight": np.ascontiguousarray(qweight),
            "xt_bf": np.ascontiguousarray(xtb_nat[pi]),
            "sb": np.ascontiguousarray(sb.reshape(-1, SW)),
            "c_mat": c_mat,
            "r1": r1,
            "e2": np.ascontiguousarray(e2b.reshape(-1, EC)),
            "a12t": np.ascontiguousarray(a12[sl][:, pi].T).astype(BF),
            "a1t": np.ascontiguousarray(a1[sl][:, pi].T).astype(BF),
        })
    return in_maps, pi


_CACHE = {}


def kernel(inp, qweight, woq_scales, woq_qzeros, woq_bias, add1, add2,
           group_size=GS, _trace=False, _repeat=1):
    from concourse import bass_utils
    inp = np.asarray(inp, dtype=np.float32)
    qweight = np.asarray(qweight, dtype=np.int32)
    woq_scales = np.asarray(woq_scales, dtype=np.float32)
    woq_qzeros = np.asarray(woq_qzeros, dtype=np.int32)
    woq_bias = np.asarray(woq_bias, dtype=np.float32)
    add1 = np.asarray(add1, dtype=np.float32)
    add2 = np.asarray(add2, dtype=np.float32)

    if "nc" not in _CACHE:
        _CACHE["nc"] = build_program()
    nc = _CACHE["nc"]
    in_maps, pi = host_prep(inp, qweight, woq_scales, woq_qzeros, woq_bias,
                            add1, add2)
    import time as _time
    times = []
    res = None
    for _ in range(max(1, _repeat)):
        t0 = _time.time()
        res = bass_utils.run_bass_kernel_spmd(
            nc, in_maps, list(range(N_CORES)), trace=_trace)
        times.append(_time.time() - t0)
    _CACHE["times"] = times
    out = np.empty((N_CORES * T_CORE, D), dtype=np.float32)
    for i in range(N_CORES):
        outt = res.results[i]["outt"]
        out[i * T_CORE:(i + 1) * T_CORE][:, pi] = outt.astype(np.float32).T
    _CACHE["last_result"] = res
    return out.reshape(inp.shape[0], inp.shape[1], D)
